# revision 9
# baseline (speedup 1.0000x reference)
"""2D DCT-II (ortho) over the last two axes of x[8, 32, 512, 512] (f32),
data-parallel across 8 NeuronCores (one batch element per core).

Quadrant decomposition: with D[u, N-1-j] = (-1)^u D[u, j], fold the image
along BOTH axes before the transform matmuls:
  colfold   T_cp[r, k]  = X[r, k] +/- X[r, 511-k]        (k < 256, all r)
  rowfold   Q_rp,cp[j,k] = T_cp[j, k] +/- T_cp[511-j, k]  (j < 256)
then with A = D[0::2, :256], B = D[1::2, :256] (M_e = A, M_o = B):
  Y[2a+rp, 2b+cp] = (M_rp Q_rp,cp M_cp^T)[a, b]
i.e. four independent 256-contraction double-matmuls — 1.5x fewer PE
row-cycles than the column-only split, and the 512x512 DT constant is
replaced by the shared 256x256 AT/BT tiles (1 MB less weight DMA).

The row fold needs rows j and 511-j on the same partition, but DMA access
patterns cannot have a negative partition step, so the bottom half of the
(column-folded) image is partition-reversed on the TensorEngine with a
constant reversal permutation REV (psum = REV.T @ U), and the DVE row-fold
reads top(SBUF) +/- reversed-bottom(PSUM).

matmul(out, lhsT, rhs) = lhsT.T @ rhs chains the stages transpose-free:
  stage 1: Z_q[k, a] = matmul(lhsT=Q_q[j, k-chunk], rhs=MT_rp[j, :])
  stage 2: Y_q[a, b] = matmul(lhsT=Z_q[k, a-chunk], rhs=MT_cp[k, :])
Matmuls run in float32r; inputs are rounded to f32r by the producing
compute ops as the BIR verifier requires.  Psum banks pair the two cp
halves (stage 1) / the output column interleave (stage 2) so each bank
drains with one 512-elem copy.
"""
import numpy as np

import concourse.bass as bass
import concourse.mybir as mybir
import concourse.tile as tile
from concourse.bass_utils import run_bass_kernel_spmd

P = 128
N = 512
H = N // 2          # 256
NIMG = 32
NCORES = 8

_MAX_WAITS = 1


def _split_excess_waits(nc):
    """walrus CoreV3 codegen rejects instructions carrying several sem
    waits; hoist excess waits onto preceding same-engine NoOps."""
    for f in nc.m.functions:
        for bb in f.blocks:
            insts = bb.instructions
            i = 0
            while i < len(insts):
                inst = insts[i]
                si = inst.sync_info
                if si is not None and si.on_wait and len(si.on_wait) > _MAX_WAITS:
                    waits = list(si.on_wait)
                    keep = waits[-_MAX_WAITS:]
                    hoist = waits[:-_MAX_WAITS]
                    nops = []
                    for w in hoist:
                        nop = mybir.InstNoOp(
                            name=nc.get_next_instruction_name(), ins=[], outs=[])
                        nop.engine = inst.engine
                        nop.sync_info = mybir.SyncInfo(on_wait=[w], on_update=[])
                        nops.append(nop)
                    si.on_wait = keep
                    for off, nop in enumerate(nops):
                        insts.insert(i + off, nop)
                    i += len(nops)
                i += 1


def _dct_mats(n=N, dtype=np.float32):
    k = np.arange(n)[:, None]
    j = np.arange(n)[None, :]
    D = np.cos(np.pi * (2 * j + 1) * k / (2.0 * n))
    D *= np.sqrt(2.0 / n)
    D[0] *= 1.0 / np.sqrt(2.0)
    D = D.astype(np.float64)
    AT = D[0::2, :H].T.astype(dtype)             # [j, a] even rows
    BT = D[1::2, :H].T.astype(dtype)             # [j, a] odd rows
    rev = np.zeros((P, P), dtype=dtype)          # REV[p, q] = 1 iff q = 127-p
    rev[np.arange(P), P - 1 - np.arange(P)] = 1.0
    return (np.ascontiguousarray(AT), np.ascontiguousarray(BT),
            np.ascontiguousarray(rev))


def _build():
    nc = bass.Bass()
    f32 = mybir.dt.float32
    f32r = mybir.dt.float32r
    x_d = nc.dram_tensor("x", [NIMG, N, N], f32, kind="ExternalInput")
    at_d = nc.dram_tensor("at", [H, H], f32, kind="ExternalInput")
    bt_d = nc.dram_tensor("bt", [H, H], f32, kind="ExternalInput")
    rev_d = nc.dram_tensor("rev", [P, P], f32, kind="ExternalInput")
    y_d = nc.dram_tensor("y", [NIMG, N, N], f32, kind="ExternalOutput")

    with tile.TileContext(nc) as tc:
        with (
            tc.tile_pool(name="const", bufs=1) as cpool,
            tc.tile_pool(name="xp", bufs=3) as xp,
            tc.tile_pool(name="xc", bufs=2) as xcp,
            tc.tile_pool(name="qp", bufs=2) as qp,
            tc.tile_pool(name="zp", bufs=2) as zp,
            tc.tile_pool(name="yp", bufs=3) as yp,
            tc.tile_pool(name="rv", bufs=1, space="PSUM") as rvp,
            tc.tile_pool(name="ps", bufs=3, space="PSUM") as ps1p,
            tc.tile_pool(name="ps2", bufs=3, space="PSUM") as ps2p,
        ):
            # consts: ab[p, jc, m, a] = MT_m[jc*128+p, a]  (m: 0=A.T, 1=B.T)
            ab_f = cpool.tile([P, 2, 2, H], f32, tag="abf")
            nc.sync.dma_start(
                ab_f[:, :, 0, :], at_d.rearrange("(jc p) a -> p jc a", p=P))
            nc.sync.dma_start(
                ab_f[:, :, 1, :], bt_d.rearrange("(jc p) a -> p jc a", p=P))
            rev_f = cpool.tile([P, P], f32, tag="revf")
            nc.sync.dma_start(rev_f[:], rev_d[:, :])
            ab_mm = cpool.tile([P, 2, 2, H], f32r, tag="abr")
            nc.scalar.copy(ab_mm[:], ab_f[:])
            rev_mm = cpool.tile([P, P], f32r, tag="revr")
            nc.scalar.copy(rev_mm[:], rev_f[:])

            for img in range(NIMG):
                # x slots i: row i*128 + p
                x_t = xp.tile([P, 4, N], f32)
                nc.sync.dma_start(
                    x_t[:], x_d[img].rearrange("(i p) c -> p i c", p=P))

                # col fold -> f32r: xc[p, cp, i, k]
                xc = xcp.tile([P, 2, 4, H], f32r)
                lo = x_t[:, :, 0:H]
                hi = x_t[:, :, N - 1:H - 1:-1]
                nc.vector.tensor_add(xc[:, 0], lo, hi)
                nc.gpsimd.tensor_sub(xc[:, 1], lo, hi)

                # partition-reverse the bottom half rows on the PE:
                # prev[q, cp, 0:256] = xc[127-q, cp, 2, :]  (row 383-q)
                # prev[q, cp, 256:512] = xc[127-q, cp, 3, :] (row 511-q)
                prev = rvp.tile([P, 2, N], f32, tag="rev")
                for cp in range(2):
                    nc.tensor.matmul(
                        prev[:, cp, :], rev_mm[:], xc[:, cp, 2:4, :],
                        start=True, stop=True)

                # row fold: q[p, rp, cp, jc, k], j = jc*128 + p
                # jc=0 pairs prev section 1 (row 511-p), jc=1 section 0
                q = qp.tile([P, 2, 2, 2, H], f32r)
                top = xc[:, :, 0:2, :]
                bot = prev[:].rearrange(
                    "p cp (sec k) -> p cp sec k", sec=2)[:, :, ::-1, :]
                nc.vector.tensor_add(q[:, 0], top, bot)
                nc.vector.tensor_sub(q[:, 1], top, bot)

                # stage 1: per (rp, kc) psum bank [128, 512] holds
                # [Z_{rp,e}[kc] | Z_{rp,o}[kc]];  Z_q[k, a]
                z = zp.tile([P, 2, 2, N], f32r)
                for rp in range(2):
                    for kc in range(2):
                        pz = ps1p.tile([P, N], f32, tag="ps1")
                        for cp in range(2):
                            for jc in range(2):
                                nc.tensor.matmul(
                                    pz[:, cp * H:(cp + 1) * H],
                                    q[:, rp, cp, jc, kc * P:(kc + 1) * P],
                                    ab_mm[:, jc, rp, :],
                                    start=(jc == 0),
                                    stop=(jc == 1),
                                )
                        if rp == 1 and kc == 1:
                            nc.vector.tensor_copy(z[:, rp, kc, :], pz[:])
                        else:
                            nc.scalar.copy(z[:, rp, kc, :], pz[:])

                # stage 2: per (rp, ac) psum bank [128, 512] holds
                # [Y_{rp,e}[ac] | Y_{rp,o}[ac]];  Y_q[a, b]
                y_t = yp.tile([P, 2, 2, N], f32)   # [p, ac, rp, c]
                for rp in range(2):
                    for ac in range(2):
                        py = ps2p.tile([P, N], f32, tag="ps2")
                        for cp in range(2):
                            for kc in range(2):
                                nc.tensor.matmul(
                                    py[:, cp * H:(cp + 1) * H],
                                    z[:, rp, kc,
                                      cp * H + ac * P:cp * H + (ac + 1) * P],
                                    ab_mm[:, kc, cp, :],
                                    start=(kc == 0),
                                    stop=(kc == 1),
                                )
                        # interleave: y[p, ac, rp, 2b+cp] = py[p, cp*256+b]
                        src_ap = py[:].rearrange("p (two k) -> p two k", two=2)
                        dst_ap = y_t[:, ac, rp, :].rearrange(
                            "p (k two) -> p two k", two=2)
                        nc.scalar.copy(dst_ap, src_ap)

                # output rows u = ac*256 + 2p + rp
                y_dst = y_d[img].rearrange("(a p b) c -> p a b c", p=P, b=2)
                for ac in range(2):
                    nc.sync.dma_start(y_dst[:, ac], y_t[:, ac])

    _split_excess_waits(nc)
    return nc


_CACHE = {}


def _get_nc():
    if "nc" not in _CACHE:
        _CACHE["nc"] = _build()
    return _CACHE["nc"]


def _in_maps(x):
    at, bt, rev = _dct_mats()
    return [{"x": x[i], "at": at, "bt": bt, "rev": rev}
            for i in range(NCORES)]


def kernel(x):
    x = np.ascontiguousarray(np.asarray(x, dtype=np.float32))
    assert x.shape == (NCORES, NIMG, N, N), x.shape
    nc = _get_nc()
    res = run_bass_kernel_spmd(nc, _in_maps(x), core_ids=list(range(NCORES)))
    out = np.stack([res.results[i]["y"] for i in range(NCORES)], axis=0)
    return out.astype(np.float32)
